# revision 1
# baseline (speedup 1.0000x reference)
"""FourierBlock kernel for 8 Trainium2 NeuronCores (Bass/Tile).

Math: only the first 64 rFFT modes survive the reference's mode
selection, so the whole block collapses to three matmul stages:

  1. forward DFT  : X[b,i,mh]   = sum_l x[b,i,l] * F[l,mh]        (mh = [Re|Im] x 64)
  2. mode mix     : Y[b,o,mh]   = per-mode complex (128x128) channel mix
  3. inverse DFT  : enh[b,o,l]  = sum_mh Y[b,o,mh] * G[mh,l]
  4. residual     : out = feature + enh

Sharding: data-parallel over batch B=32 across 8 cores (4 samples each);
the DFT bases and mode-mix weights are replicated (passed pre-converted
to bf16 from the host). All matmuls run in bf16 with fp32 PSUM
accumulation; the residual add is fp32.  The `enhance` term is ~1e-4 of
`feature` in norm, so bf16 stage error (~0.5%) dilutes to ~4e-7 in the
output.

Stage-2 layout trick: per-mode weights W_m [i,o] are the PE stationary
operand; the moving operand packs [Xr_m | Xi_m] (and [-Xi_m | Xr_m])
for all 4 local samples into 8 columns, so each mode costs exactly two
matmuls accumulating Yr|Yi into one PSUM bank.
"""

import sys

import numpy as np

try:
    import concourse.bass as bass
except ImportError:  # containers keep the repo at /opt/trn_rl_repo
    sys.path.insert(0, "/opt/trn_rl_repo")
    import concourse.bass as bass

import concourse.mybir as mybir
import concourse.tile as tile
from concourse.bass_utils import run_bass_kernel_spmd

import ml_dtypes

BF16 = mybir.dt.bfloat16
F32 = mybir.dt.float32

B, C, H, W = 32, 128, 64, 64
L = H * W                      # 4096
MODES = 64
NCORES = 8
BS = B // NCORES               # 4 samples per core
LC = L // 128                  # 32 l-chunks of 128


def build_program():
    """Build the per-core Bass program (same program on all 8 cores)."""
    nc = bass.Bass()

    xt_d = nc.dram_tensor("xt", [128, LC, BS * 128], BF16, kind="ExternalInput")
    xf_d = nc.dram_tensor("xf", [BS, 128, L], F32, kind="ExternalInput")
    wr_d = nc.dram_tensor("wr", [128, MODES, 128], BF16, kind="ExternalInput")
    wi_d = nc.dram_tensor("wi", [128, MODES, 128], BF16, kind="ExternalInput")
    fb_d = nc.dram_tensor("fb", [128, LC, 128], BF16, kind="ExternalInput")
    gb_d = nc.dram_tensor("gb", [128, L], BF16, kind="ExternalInput")
    id_d = nc.dram_tensor("idn", [128, 128], BF16, kind="ExternalInput")
    out_d = nc.dram_tensor("out", [BS, 128, L], F32, kind="ExternalOutput")

    with tile.TileContext(nc) as tc:
        with (
            tc.tile_pool(name="consts", bufs=1) as consts,
            tc.tile_pool(name="xstage", bufs=1) as xstage,
            tc.tile_pool(name="outp", bufs=4) as outp,
            tc.tile_pool(name="ps_x", bufs=2, space="PSUM") as ps_x,
            tc.tile_pool(name="ps_y", bufs=1, space="PSUM") as ps_y,
            tc.tile_pool(name="ps_t", bufs=2, space="PSUM") as ps_t,
            tc.tile_pool(name="ps_e", bufs=2, space="PSUM") as ps_e,
        ):
            # ---- resident inputs -------------------------------------
            xt = consts.tile([128, LC, BS * 128], BF16)   # x^T blocks [l, (b,i)]
            fb = consts.tile([128, LC, 128], BF16)        # DFT basis chunks
            wr = consts.tile([128, MODES, 128], BF16)
            wi = consts.tile([128, MODES, 128], BF16)
            gb = consts.tile([128, L], BF16)              # inverse-DFT basis
            idn = consts.tile([128, 128], BF16)
            xf = consts.tile([128, BS, L], F32)           # fp32 x for residual

            nc.sync.dma_start(xt[:], xt_d[:])
            nc.sync.dma_start(fb[:], fb_d[:])
            nc.sync.dma_start(wr[:], wr_d[:])
            nc.sync.dma_start(wi[:], wi_d[:])
            nc.sync.dma_start(gb[:], gb_d[:])
            nc.sync.dma_start(idn[:], id_d[:])
            for b in range(BS):
                nc.sync.dma_start(xf[:, b, :], xf_d[b])

            # ---- stage 2 moving operands ------------------------------
            # xcat  cols m*8+k : k<4 -> Xr[b=k],   k>=4 -> Xi[b=k-4]
            # xcat2 cols m*8+k : k<4 -> -Xi[b=k],  k>=4 -> Xr[b=k-4]
            xcat = xstage.tile([128, MODES, 8], BF16)
            xcat2 = xstage.tile([128, MODES, 8], BF16)
            # ycat [o, b, mh] bf16 staging for the Y transposes
            ycat = xstage.tile([128, BS, 128], BF16)
            # yt [mh, b, o] bf16 = stage-3 stationary operand
            yt = xstage.tile([128, BS, 128], BF16)

            # ---- stage 1: forward DFT, per local sample --------------
            for b in range(BS):
                xps = ps_x.tile([128, 128], F32, tag="xps")
                for lc in range(LC):
                    nc.tensor.matmul(
                        xps[:],
                        xt[:, lc, b * 128:(b + 1) * 128],
                        fb[:, lc, :],
                        start=(lc == 0),
                        stop=(lc == LC - 1),
                    )
                # drain [i, 128] psum -> bf16 stage-2 operand layouts
                nc.vector.tensor_copy(xcat[:, :, b], xps[:, 0:MODES])        # Xr
                nc.vector.tensor_copy(xcat[:, :, 4 + b], xps[:, MODES:128])  # Xi
                nc.vector.tensor_copy(xcat2[:, :, 4 + b], xps[:, 0:MODES])   # Xr
                nc.scalar.mul(xcat2[:, :, b], xps[:, MODES:128], -1.0)       # -Xi

            # ---- stage 2: per-mode complex channel mix ---------------
            # ycomb cols m*8+k : k<4 -> Yr[b=k], k>=4 -> Yi[b=k-4]
            ycomb = ps_y.tile([128, MODES, 8], F32)
            for m in range(MODES):
                # Yr = Wr@Xr - Wi@Xi ; Yi = Wr@Xi + Wi@Xr
                nc.tensor.matmul(
                    ycomb[:, m, :], wr[:, m, :], xcat[:, m, :],
                    start=True, stop=False,
                )
                nc.tensor.matmul(
                    ycomb[:, m, :], wi[:, m, :], xcat2[:, m, :],
                    start=False, stop=True,
                )

            # ---- Y -> [mh, o] per sample (PE transpose) --------------
            for b in range(BS):
                nc.vector.tensor_copy(ycat[:, b, 0:MODES], ycomb[:, :, b])
                nc.vector.tensor_copy(ycat[:, b, MODES:128], ycomb[:, :, 4 + b])
            for b in range(BS):
                pst = ps_t.tile([128, 128], BF16, tag="pst")
                nc.tensor.transpose(pst[:], ycat[:, b, :], idn[:])
                nc.vector.tensor_copy(yt[:, b, :], pst[:])

            # ---- stage 3: inverse DFT + residual add -----------------
            for b in range(BS):
                for n in range(L // 512):
                    enh = ps_e.tile([128, 512], F32, tag="enh")
                    nc.tensor.matmul(
                        enh[:], yt[:, b, :], gb[:, n * 512:(n + 1) * 512],
                        start=True, stop=True,
                    )
                    ob = outp.tile([128, 512], F32, tag="ob")
                    nc.vector.tensor_add(
                        ob[:], enh[:], xf[:, b, n * 512:(n + 1) * 512]
                    )
                    nc.sync.dma_start(out_d[b][:, n * 512:(n + 1) * 512], ob[:])

    split_multi_waits(nc)
    return nc


def split_multi_waits(nc, max_waits=1):
    """The walrus in this container only accepts one sync-wait per
    instruction; move extras onto injected NoOps on the same engine."""
    cnt = 0
    for fn in nc.m.functions:
        for bb in fn.blocks:
            out = []
            for inst in bb.instructions:
                si = inst.sync_info
                if si is not None and si.on_wait and len(si.on_wait) > max_waits:
                    waits = list(si.on_wait)
                    for w in waits[:-max_waits]:
                        cnt += 1
                        nop = mybir.InstNoOp(
                            name=f"xsplitwait_{cnt}", ins=[], outs=[],
                            sync_info=mybir.SyncInfo(on_wait=[w], on_update=[]),
                        )
                        nop.engine = inst.engine
                        out.append(nop)
                    si.on_wait = waits[-max_waits:]
                    inst.sync_info = si
                out.append(inst)
            bb.instructions[:] = out
    return cnt


def make_host_inputs(feature, weights1_real, weights1_img):
    """Host-side prep: DFT bases, weight re-layout, per-core shards."""
    bf16 = ml_dtypes.bfloat16
    x = np.ascontiguousarray(feature.reshape(B, C, L))

    lv = np.arange(L, dtype=np.float64)
    mv = np.arange(MODES, dtype=np.float64)
    theta = 2.0 * np.pi * np.outer(lv, mv) / L            # (L, 64)
    F = np.concatenate([np.cos(theta), -np.sin(theta)], axis=1)   # (L, 128)
    a = np.full(MODES, 2.0 / L)
    a[0] = 1.0 / L
    G = np.concatenate(
        [a[:, None] * np.cos(theta.T), -(a[:, None]) * np.sin(theta.T)], axis=0
    )                                                     # (128, L)

    # fb[p, c, m] = F[c*128+p, m]
    fb = np.ascontiguousarray(
        F.reshape(LC, 128, 128).transpose(1, 0, 2)
    ).astype(bf16)
    gb = np.ascontiguousarray(G).astype(bf16)

    # wr/wi[i, m, o] from (i, o, m, 1)
    wr = np.ascontiguousarray(
        weights1_real[..., 0].transpose(0, 2, 1)
    ).astype(bf16)
    wi = np.ascontiguousarray(
        weights1_img[..., 0].transpose(0, 2, 1)
    ).astype(bf16)

    idn = np.eye(128, dtype=np.float32).astype(bf16)

    in_maps = []
    for c in range(NCORES):
        xc = x[c * BS:(c + 1) * BS]                       # (4, 128, L)
        # xt[p, lc, b*128+i] = x[b, i, lc*128+p]
        xtc = xc.transpose(2, 0, 1).reshape(L, BS * 128)  # (L, 512)
        xt = np.ascontiguousarray(
            xtc.reshape(LC, 128, BS * 128).transpose(1, 0, 2)
        ).astype(bf16)
        in_maps.append({
            "xt": xt,
            "xf": np.ascontiguousarray(xc, dtype=np.float32),
            "wr": wr,
            "wi": wi,
            "fb": fb,
            "gb": gb,
            "idn": idn,
        })
    return in_maps


_CACHE = {}


def get_program():
    if "nc" not in _CACHE:
        _CACHE["nc"] = build_program()
    return _CACHE["nc"]


def kernel(feature, weights1_real, weights1_img):
    feature = np.asarray(feature, dtype=np.float32)
    weights1_real = np.asarray(weights1_real, dtype=np.float32)
    weights1_img = np.asarray(weights1_img, dtype=np.float32)

    nc = get_program()
    in_maps = make_host_inputs(feature, weights1_real, weights1_img)
    res = run_bass_kernel_spmd(nc, in_maps, list(range(NCORES)))
    out = np.concatenate([res.results[c]["out"] for c in range(NCORES)], axis=0)
    return out.reshape(B, C, H, W).astype(np.float32)


# revision 4
# speedup vs baseline: 1248.9073x; 1248.9073x over previous
"""FourierBlock kernel for 8 Trainium2 NeuronCores (Bass/Tile).

Math: only the first 64 rFFT modes survive the reference's mode
selection, so the whole block collapses to three matmul stages:

  1. forward DFT  : X[b,i,mh]   = sum_l x[b,i,l] * F[l,mh]        (mh = [Re|Im] x 64)
  2. mode mix     : Y[b,o,mh]   = per-mode complex (128x128) channel mix
  3. inverse DFT  : enh[b,o,l]  = sum_mh Y[b,o,mh] * G[mh,l]
  4. residual     : out = feature + enh

Sharding: data-parallel over batch B=32 across 8 cores (4 samples each);
the DFT bases and mode-mix weights are replicated (passed pre-converted
to bf16 from the host). All matmuls run in bf16 with fp32 PSUM
accumulation; the residual add is fp32.  The `enhance` term is ~1e-4 of
`feature` in norm, so bf16 stage error (~0.5%) dilutes to ~4e-7 in the
output.

Stage-2 layout trick: per-mode weights W_m [i,o] are the PE stationary
operand; the moving operand packs [Xr_m | Xi_m] (and [-Xi_m | Xr_m])
for all 4 local samples into 8 columns, so each mode costs exactly two
matmuls accumulating Yr|Yi into one PSUM bank.
"""

import sys

import numpy as np

try:
    import concourse.bass as bass
except ImportError:  # containers keep the repo at /opt/trn_rl_repo
    sys.path.insert(0, "/opt/trn_rl_repo")
    import concourse.bass as bass

import concourse.mybir as mybir
import concourse.tile as tile
from concourse.bass_utils import run_bass_kernel_spmd

import ml_dtypes

BF16 = mybir.dt.bfloat16
F32 = mybir.dt.float32

B, C, H, W = 32, 128, 64, 64
L = H * W                      # 4096
MODES = 64
NCORES = 8
BS = B // NCORES               # 4 samples per core
LC = L // 128                  # 32 l-chunks of 128


def build_program(reps=1):
    """Build the per-core Bass program (same program on all 8 cores).

    reps>1 repeats the whole pipeline (same inputs/outputs) so device
    time per rep can be measured as a wall-clock delta, amortizing the
    axon dispatch overhead."""
    nc = bass.Bass()

    xt_d = nc.dram_tensor("xt", [128, LC, BS * 128], BF16, kind="ExternalInput")
    xf_d = nc.dram_tensor("xf", [BS, 128, L], F32, kind="ExternalInput")
    wr_d = nc.dram_tensor("wr", [128, MODES, 128], BF16, kind="ExternalInput")
    wi_d = nc.dram_tensor("wi", [128, MODES, 128], BF16, kind="ExternalInput")
    fb_d = nc.dram_tensor("fb", [128, LC, 128], BF16, kind="ExternalInput")
    gb_d = nc.dram_tensor("gb", [128, L], BF16, kind="ExternalInput")
    id_d = nc.dram_tensor("idn", [128, 128], BF16, kind="ExternalInput")
    out_d = nc.dram_tensor("out", [BS, 128, L], F32, kind="ExternalOutput")

    with tile.TileContext(nc) as tc:
        with (
            tc.tile_pool(name="consts", bufs=1) as consts,
            tc.tile_pool(name="xstage", bufs=2) as xstage,
            tc.tile_pool(name="outp", bufs=4) as outp,
            tc.tile_pool(name="ps_x", bufs=2, space="PSUM") as ps_x,
            tc.tile_pool(name="ps_y", bufs=2, space="PSUM") as ps_y,
            tc.tile_pool(name="ps_t", bufs=2, space="PSUM") as ps_t,
            tc.tile_pool(name="ps_e", bufs=2, space="PSUM") as ps_e,
        ):
            # ---- resident inputs -------------------------------------
            xt = consts.tile([128, LC, BS * 128], BF16)   # x^T blocks [l, (b,i)]
            fb = consts.tile([128, LC, 128], BF16)        # DFT basis chunks
            wr = consts.tile([128, MODES, 128], BF16)
            wi = consts.tile([128, MODES, 128], BF16)
            gb = consts.tile([128, L], BF16)              # inverse-DFT basis
            idn = consts.tile([128, 128], BF16)
            xf = consts.tile([128, BS, L], F32)           # fp32 x for residual

            nc.sync.dma_start(xt[:], xt_d[:])
            nc.sync.dma_start(fb[:], fb_d[:])
            nc.sync.dma_start(wr[:], wr_d[:])
            nc.sync.dma_start(wi[:], wi_d[:])
            nc.sync.dma_start(gb[:], gb_d[:])
            nc.sync.dma_start(idn[:], id_d[:])
            for b in range(BS):
                nc.sync.dma_start(xf[:, b, :], xf_d[b])

            for _rep in range(reps):
                # ---- stage 2 moving operands --------------------------
                # xcat  cols m*8+k : k<4 -> Xr[b=k],   k>=4 -> Xi[b=k-4]
                # xcat2 cols m*8+k : k<4 -> -Xi[b=k],  k>=4 -> Xr[b=k-4]
                xcat = xstage.tile([128, MODES, 8], BF16, tag="xcat")
                xcat2 = xstage.tile([128, MODES, 8], BF16, tag="xcat2")
                # ycat [o, b, mh] bf16 staging for the Y transposes
                ycat = xstage.tile([128, BS, 128], BF16, tag="ycat")
                # yt [mh, b, o] bf16 = stage-3 stationary operand
                yt = xstage.tile([128, BS, 128], BF16, tag="yt")

                # ---- stage 1: forward DFT, per local sample ----------
                for b in range(BS):
                    xps = ps_x.tile([128, 128], F32, tag="xps")
                    for lc in range(LC):
                        nc.tensor.matmul(
                            xps[:],
                            xt[:, lc, b * 128:(b + 1) * 128],
                            fb[:, lc, :],
                            start=(lc == 0),
                            stop=(lc == LC - 1),
                        )
                    # drain [i, 128] psum -> bf16 stage-2 operand layouts
                    nc.vector.tensor_copy(xcat[:, :, b], xps[:, 0:MODES])
                    nc.vector.tensor_copy(xcat[:, :, 4 + b], xps[:, MODES:128])
                    nc.vector.tensor_copy(xcat2[:, :, 4 + b], xps[:, 0:MODES])
                    nc.scalar.mul(xcat2[:, :, b], xps[:, MODES:128], -1.0)

                # ---- stage 2: per-mode complex channel mix -----------
                # ycomb cols m*8+k : k<4 -> Yr[b=k], k>=4 -> Yi[b=k-4]
                ycomb = ps_y.tile([128, MODES, 8], F32, tag="ycomb")
                for m in range(MODES):
                    # Yr = Wr@Xr - Wi@Xi ; Yi = Wr@Xi + Wi@Xr
                    nc.tensor.matmul(
                        ycomb[:, m, :], wr[:, m, :], xcat[:, m, :],
                        start=True, stop=False,
                    )
                    nc.tensor.matmul(
                        ycomb[:, m, :], wi[:, m, :], xcat2[:, m, :],
                        start=False, stop=True,
                    )

                # ---- Y -> [mh, o] per sample (PE transpose) ----------
                for b in range(BS):
                    nc.vector.tensor_copy(ycat[:, b, 0:MODES], ycomb[:, :, b])
                    nc.vector.tensor_copy(ycat[:, b, MODES:128],
                                          ycomb[:, :, 4 + b])
                for b in range(BS):
                    pst = ps_t.tile([128, 128], BF16, tag="pst")
                    nc.tensor.transpose(pst[:], ycat[:, b, :], idn[:])
                    nc.vector.tensor_copy(yt[:, b, :], pst[:])

                # ---- stage 3: inverse DFT + residual add -------------
                for b in range(BS):
                    for n in range(L // 512):
                        enh = ps_e.tile([128, 512], F32, tag="enh")
                        nc.tensor.matmul(
                            enh[:], yt[:, b, :], gb[:, n * 512:(n + 1) * 512],
                            start=True, stop=True,
                        )
                        ob = outp.tile([128, 512], F32, tag="ob")
                        nc.vector.tensor_add(
                            ob[:], enh[:], xf[:, b, n * 512:(n + 1) * 512]
                        )
                        nc.sync.dma_start(
                            out_d[b][:, n * 512:(n + 1) * 512], ob[:])

    split_multi_waits(nc)
    return nc


def split_multi_waits(nc, max_waits=1):
    """The walrus in this container only accepts one sync-wait per
    instruction; move extras onto injected NoOps on the same engine."""
    cnt = 0
    for fn in nc.m.functions:
        for bb in fn.blocks:
            out = []
            for inst in bb.instructions:
                si = inst.sync_info
                if si is not None and si.on_wait and len(si.on_wait) > max_waits:
                    waits = list(si.on_wait)
                    for w in waits[:-max_waits]:
                        cnt += 1
                        nop = mybir.InstNoOp(
                            name=f"xsplitwait_{cnt}", ins=[], outs=[],
                            sync_info=mybir.SyncInfo(on_wait=[w], on_update=[]),
                        )
                        nop.engine = inst.engine
                        out.append(nop)
                    si.on_wait = waits[-max_waits:]
                    inst.sync_info = si
                out.append(inst)
            bb.instructions[:] = out
    return cnt


def make_host_inputs(feature, weights1_real, weights1_img):
    """Host-side prep: DFT bases, weight re-layout, per-core shards."""
    bf16 = ml_dtypes.bfloat16
    x = np.ascontiguousarray(feature.reshape(B, C, L))

    lv = np.arange(L, dtype=np.float64)
    mv = np.arange(MODES, dtype=np.float64)
    theta = 2.0 * np.pi * np.outer(lv, mv) / L            # (L, 64)
    F = np.concatenate([np.cos(theta), -np.sin(theta)], axis=1)   # (L, 128)
    a = np.full(MODES, 2.0 / L)
    a[0] = 1.0 / L
    G = np.concatenate(
        [a[:, None] * np.cos(theta.T), -(a[:, None]) * np.sin(theta.T)], axis=0
    )                                                     # (128, L)

    # fb[p, c, m] = F[c*128+p, m]
    fb = np.ascontiguousarray(
        F.reshape(LC, 128, 128).transpose(1, 0, 2)
    ).astype(bf16)
    gb = np.ascontiguousarray(G).astype(bf16)

    # wr/wi[i, m, o] from (i, o, m, 1)
    wr = np.ascontiguousarray(
        weights1_real[..., 0].transpose(0, 2, 1)
    ).astype(bf16)
    wi = np.ascontiguousarray(
        weights1_img[..., 0].transpose(0, 2, 1)
    ).astype(bf16)

    idn = np.eye(128, dtype=np.float32).astype(bf16)

    in_maps = []
    for c in range(NCORES):
        xc = x[c * BS:(c + 1) * BS]                       # (4, 128, L)
        # xt[p, lc, b*128+i] = x[b, i, lc*128+p]
        xtc = xc.transpose(2, 0, 1).reshape(L, BS * 128)  # (L, 512)
        xt = np.ascontiguousarray(
            xtc.reshape(LC, 128, BS * 128).transpose(1, 0, 2)
        ).astype(bf16)
        in_maps.append({
            "xt": xt,
            "xf": np.ascontiguousarray(xc, dtype=np.float32),
            "wr": wr,
            "wi": wi,
            "fb": fb,
            "gb": gb,
            "idn": idn,
        })
    return in_maps


_CACHE = {}


def get_program(reps=1):
    key = ("nc", reps)
    if key not in _CACHE:
        _CACHE[key] = build_program(reps)
    return _CACHE[key]


def kernel(feature, weights1_real, weights1_img):
    feature = np.asarray(feature, dtype=np.float32)
    weights1_real = np.asarray(weights1_real, dtype=np.float32)
    weights1_img = np.asarray(weights1_img, dtype=np.float32)

    nc = get_program()
    in_maps = make_host_inputs(feature, weights1_real, weights1_img)
    res = run_bass_kernel_spmd(nc, in_maps, list(range(NCORES)))
    out = np.concatenate([res.results[c]["out"] for c in range(NCORES)], axis=0)
    return out.reshape(B, C, H, W).astype(np.float32)


# revision 5
# speedup vs baseline: 1568.4349x; 1.2558x over previous
"""FourierBlock kernel for 8 Trainium2 NeuronCores (Bass/Tile).

Math: only the first 64 rFFT modes survive the reference's mode
selection, so the whole block collapses to three matmul stages:

  1. forward DFT  : X[b,i,mh]   = sum_l x[b,i,l] * F[l,mh]        (mh = [Re|Im] x 64)
  2. mode mix     : Y[b,o,mh]   = per-mode complex (128x128) channel mix
  3. inverse DFT  : enh[b,o,l]  = sum_mh Y[b,o,mh] * G[mh,l]
  4. residual     : out = feature + enh

Sharding: data-parallel over batch B=32 across 8 cores (4 samples each);
the DFT bases and mode-mix weights are replicated (passed pre-converted
to bf16 from the host). All matmuls run in bf16 with fp32 PSUM
accumulation; the residual add is fp32.  The `enhance` term is ~1e-4 of
`feature` in norm, so bf16 stage error (~0.5%) dilutes to ~4e-7 in the
output.

Stage-2 layout trick: per-mode weights W_m [i,o] are the PE stationary
operand; the moving operand packs [Xr_m | Xi_m] (and [-Xi_m | Xr_m])
for all 4 local samples into 8 columns, so each mode costs exactly two
matmuls accumulating Yr|Yi into one PSUM bank.
"""

import sys

import numpy as np

try:
    import concourse.bass as bass
except ImportError:  # containers keep the repo at /opt/trn_rl_repo
    sys.path.insert(0, "/opt/trn_rl_repo")
    import concourse.bass as bass

import concourse.mybir as mybir
import concourse.tile as tile
from concourse.bass_utils import run_bass_kernel_spmd

import ml_dtypes

BF16 = mybir.dt.bfloat16
F32 = mybir.dt.float32

B, C, H, W = 32, 128, 64, 64
L = H * W                      # 4096
MODES = 64
NCORES = 8
BS = B // NCORES               # 4 samples per core
LC = L // 128                  # 32 l-chunks of 128


def build_program(reps=1):
    """Build the per-core Bass program (same program on all 8 cores).

    reps>1 repeats the whole pipeline (same inputs/outputs) so device
    time per rep can be measured as a wall-clock delta, amortizing the
    axon dispatch overhead."""
    nc = bass.Bass()

    xt_d = nc.dram_tensor("xt", [128, LC, BS * 128], BF16, kind="ExternalInput")
    xf_d = nc.dram_tensor("xf", [BS, 128, L], F32, kind="ExternalInput")
    wr_d = nc.dram_tensor("wr", [128, MODES, 128], BF16, kind="ExternalInput")
    wi_d = nc.dram_tensor("wi", [128, MODES, 128], BF16, kind="ExternalInput")
    fb_d = nc.dram_tensor("fb", [128, LC, 128], BF16, kind="ExternalInput")
    gb_d = nc.dram_tensor("gb", [128, L], BF16, kind="ExternalInput")
    id_d = nc.dram_tensor("idn", [128, 128], BF16, kind="ExternalInput")
    out_d = nc.dram_tensor("out", [BS, 128, L], F32, kind="ExternalOutput")

    with tile.TileContext(nc) as tc:
        with (
            tc.tile_pool(name="consts", bufs=1) as consts,
            tc.tile_pool(name="xstage", bufs=2) as xstage,
            tc.tile_pool(name="outp", bufs=4) as outp,
            tc.tile_pool(name="ps_x", bufs=2, space="PSUM") as ps_x,
            tc.tile_pool(name="ps_y", bufs=2, space="PSUM") as ps_y,
            tc.tile_pool(name="ps_t", bufs=2, space="PSUM") as ps_t,
            tc.tile_pool(name="ps_e", bufs=2, space="PSUM") as ps_e,
        ):
            for _rep in range(reps):
                # ---- inputs (loaded per rep so reps-delta timing is
                # faithful to the single-shot kernel) -------------------
                xt = consts.tile([128, LC, BS * 128], BF16, tag="xt")
                fb = consts.tile([128, LC, 128], BF16, tag="fb")
                wr = consts.tile([128, MODES, 128], BF16, tag="wr")
                wi = consts.tile([128, MODES, 128], BF16, tag="wi")
                gb = consts.tile([128, L], BF16, tag="gb")
                idn = consts.tile([128, 128], BF16, tag="idn")
                xf = consts.tile([128, BS, L], F32, tag="xf")

                nc.sync.dma_start(xt[:], xt_d[:])
                nc.sync.dma_start(fb[:], fb_d[:])
                nc.sync.dma_start(wr[:], wr_d[:])
                nc.sync.dma_start(wi[:], wi_d[:])
                nc.sync.dma_start(gb[:], gb_d[:])
                nc.sync.dma_start(idn[:], id_d[:])
                for b in range(BS):
                    nc.sync.dma_start(xf[:, b, :], xf_d[b])
                # ---- stage 2 moving operands --------------------------
                # xcat  cols m*8+k : k<4 -> Xr[b=k],   k>=4 -> Xi[b=k-4]
                # xcat2 cols m*8+k : k<4 -> -Xi[b=k],  k>=4 -> Xr[b=k-4]
                xcat = xstage.tile([128, MODES, 8], BF16, tag="xcat")
                xcat2 = xstage.tile([128, MODES, 8], BF16, tag="xcat2")
                # ycat [o, b, mh] bf16 staging for the Y transposes
                ycat = xstage.tile([128, BS, 128], BF16, tag="ycat")
                # yt [mh, b, o] bf16 = stage-3 stationary operand
                yt = xstage.tile([128, BS, 128], BF16, tag="yt")

                # ---- stage 1: forward DFT, per local sample ----------
                for b in range(BS):
                    xps = ps_x.tile([128, 128], F32, tag="xps")
                    for lc in range(LC):
                        nc.tensor.matmul(
                            xps[:],
                            xt[:, lc, b * 128:(b + 1) * 128],
                            fb[:, lc, :],
                            start=(lc == 0),
                            stop=(lc == LC - 1),
                        )
                    # drain [i, 128] psum -> bf16 stage-2 operand layouts
                    nc.vector.tensor_copy(xcat[:, :, b], xps[:, 0:MODES])
                    nc.vector.tensor_copy(xcat[:, :, 4 + b], xps[:, MODES:128])
                    nc.vector.tensor_copy(xcat2[:, :, 4 + b], xps[:, 0:MODES])
                    nc.scalar.mul(xcat2[:, :, b], xps[:, MODES:128], -1.0)

                # ---- stage 2: per-mode complex channel mix -----------
                # ycomb cols m*8+k : k<4 -> Yr[b=k], k>=4 -> Yi[b=k-4]
                ycomb = ps_y.tile([128, MODES, 8], F32, tag="ycomb")
                for m in range(MODES):
                    # Yr = Wr@Xr - Wi@Xi ; Yi = Wr@Xi + Wi@Xr
                    nc.tensor.matmul(
                        ycomb[:, m, :], wr[:, m, :], xcat[:, m, :],
                        start=True, stop=False,
                    )
                    nc.tensor.matmul(
                        ycomb[:, m, :], wi[:, m, :], xcat2[:, m, :],
                        start=False, stop=True,
                    )

                # ---- Y -> [mh, o] per sample (PE transpose) ----------
                for b in range(BS):
                    nc.vector.tensor_copy(ycat[:, b, 0:MODES], ycomb[:, :, b])
                    nc.vector.tensor_copy(ycat[:, b, MODES:128],
                                          ycomb[:, :, 4 + b])
                for b in range(BS):
                    pst = ps_t.tile([128, 128], BF16, tag="pst")
                    nc.tensor.transpose(pst[:], ycat[:, b, :], idn[:])
                    nc.vector.tensor_copy(yt[:, b, :], pst[:])

                # ---- stage 3: inverse DFT + residual add -------------
                for b in range(BS):
                    for n in range(L // 512):
                        enh = ps_e.tile([128, 512], F32, tag="enh")
                        nc.tensor.matmul(
                            enh[:], yt[:, b, :], gb[:, n * 512:(n + 1) * 512],
                            start=True, stop=True,
                        )
                        ob = outp.tile([128, 512], F32, tag="ob")
                        nc.vector.tensor_add(
                            ob[:], enh[:], xf[:, b, n * 512:(n + 1) * 512]
                        )
                        nc.sync.dma_start(
                            out_d[b][:, n * 512:(n + 1) * 512], ob[:])

    split_multi_waits(nc)
    return nc


def split_multi_waits(nc, max_waits=1):
    """The walrus in this container only accepts one sync-wait per
    instruction; move extras onto injected NoOps on the same engine."""
    cnt = 0
    for fn in nc.m.functions:
        for bb in fn.blocks:
            out = []
            for inst in bb.instructions:
                si = inst.sync_info
                if si is not None and si.on_wait and len(si.on_wait) > max_waits:
                    waits = list(si.on_wait)
                    for w in waits[:-max_waits]:
                        cnt += 1
                        nop = mybir.InstNoOp(
                            name=f"xsplitwait_{cnt}", ins=[], outs=[],
                            sync_info=mybir.SyncInfo(on_wait=[w], on_update=[]),
                        )
                        nop.engine = inst.engine
                        out.append(nop)
                    si.on_wait = waits[-max_waits:]
                    inst.sync_info = si
                out.append(inst)
            bb.instructions[:] = out
    return cnt


def make_host_inputs(feature, weights1_real, weights1_img):
    """Host-side prep: DFT bases, weight re-layout, per-core shards."""
    bf16 = ml_dtypes.bfloat16
    x = np.ascontiguousarray(feature.reshape(B, C, L))

    lv = np.arange(L, dtype=np.float64)
    mv = np.arange(MODES, dtype=np.float64)
    theta = 2.0 * np.pi * np.outer(lv, mv) / L            # (L, 64)
    F = np.concatenate([np.cos(theta), -np.sin(theta)], axis=1)   # (L, 128)
    a = np.full(MODES, 2.0 / L)
    a[0] = 1.0 / L
    G = np.concatenate(
        [a[:, None] * np.cos(theta.T), -(a[:, None]) * np.sin(theta.T)], axis=0
    )                                                     # (128, L)

    # fb[p, c, m] = F[c*128+p, m]
    fb = np.ascontiguousarray(
        F.reshape(LC, 128, 128).transpose(1, 0, 2)
    ).astype(bf16)
    gb = np.ascontiguousarray(G).astype(bf16)

    # wr/wi[i, m, o] from (i, o, m, 1)
    wr = np.ascontiguousarray(
        weights1_real[..., 0].transpose(0, 2, 1)
    ).astype(bf16)
    wi = np.ascontiguousarray(
        weights1_img[..., 0].transpose(0, 2, 1)
    ).astype(bf16)

    idn = np.eye(128, dtype=np.float32).astype(bf16)

    in_maps = []
    for c in range(NCORES):
        xc = x[c * BS:(c + 1) * BS]                       # (4, 128, L)
        # xt[p, lc, b*128+i] = x[b, i, lc*128+p]
        xtc = xc.transpose(2, 0, 1).reshape(L, BS * 128)  # (L, 512)
        xt = np.ascontiguousarray(
            xtc.reshape(LC, 128, BS * 128).transpose(1, 0, 2)
        ).astype(bf16)
        in_maps.append({
            "xt": xt,
            "xf": np.ascontiguousarray(xc, dtype=np.float32),
            "wr": wr,
            "wi": wi,
            "fb": fb,
            "gb": gb,
            "idn": idn,
        })
    return in_maps


_CACHE = {}


def get_program(reps=1):
    key = ("nc", reps)
    if key not in _CACHE:
        _CACHE[key] = build_program(reps)
    return _CACHE[key]


def kernel(feature, weights1_real, weights1_img):
    feature = np.asarray(feature, dtype=np.float32)
    weights1_real = np.asarray(weights1_real, dtype=np.float32)
    weights1_img = np.asarray(weights1_img, dtype=np.float32)

    nc = get_program()
    in_maps = make_host_inputs(feature, weights1_real, weights1_img)
    res = run_bass_kernel_spmd(nc, in_maps, list(range(NCORES)))
    out = np.concatenate([res.results[c]["out"] for c in range(NCORES)], axis=0)
    return out.reshape(B, C, H, W).astype(np.float32)


# revision 11
# speedup vs baseline: 2825.5626x; 1.8015x over previous
"""FourierBlock kernel for 8 Trainium2 NeuronCores (Bass/Tile).

Math: only the first 64 rFFT modes survive the reference's mode
selection, so the whole block collapses to three matmul stages:

  1. forward DFT  : X[b,i,mh]   = sum_l x[b,i,l] * F[l,mh]        (mh = [Re|Im] x 64)
  2. mode mix     : Y[b,o,mh]   = per-mode complex (128x128) channel mix
  3. inverse DFT  : enh[b,o,l]  = sum_mh Y[b,o,mh] * G[mh,l]
  4. residual     : out = feature + enh

Sharding: data-parallel over batch B=32 across 8 cores (4 samples each);
the DFT bases and mode-mix weights are replicated (pre-converted on the
host). Stages 1-3 run in fp8e3 (e3m4) with fp32 PSUM accumulation and
power-of-two scale folding (SF_*) to keep every operand in fp8 range;
the residual add is exact fp32.  The `enhance` term is ~1e-4 of
`feature` in norm, so the ~3% fp8 stage error dilutes to ~2e-5 in the
output.  The kernel is DMA-bound: ~13 MiB in + 8 MiB out per core.

Stage-2 layout trick: per-mode weights W_m [i,o] are the PE stationary
operand; the moving operand packs [Xr_m | Xi_m] (and [-Xi_m | Xr_m])
for all 4 local samples into 8 columns, so each mode costs exactly two
matmuls accumulating Yr|Yi into one PSUM bank.
"""

import sys

import numpy as np

try:
    import concourse.bass as bass
except ImportError:  # containers keep the repo at /opt/trn_rl_repo
    sys.path.insert(0, "/opt/trn_rl_repo")
    import concourse.bass as bass

import concourse.mybir as mybir
import concourse.tile as tile
from concourse.bass_utils import run_bass_kernel_spmd

import ml_dtypes

BF16 = mybir.dt.bfloat16
FP8 = mybir.dt.float8e3          # e3m4: rel step ~3%, range +-15.5
F32 = mybir.dt.float32

USE_FP8 = True                   # fp8 stages 1-3 (halves their DMA bytes)
SF_X = (1.0 / 32.0) if USE_FP8 else 1.0   # folded into fb: |X| in fp8 range
SF_W = (2.0 ** 17) if USE_FP8 else 1.0    # lifts weights into fp8 normals
SF_Y = (2.0 ** -9) if USE_FP8 else 1.0    # ycat drain scale: |yt| in fp8 range
SF_G = (2.0 ** 14) if USE_FP8 else 1.0    # lifts gb into fp8 normals
SF_OUT = 1.0 / (SF_X * SF_W * SF_Y * SF_G)  # descale in the residual STT


B, C, H, W = 32, 128, 64, 64
L = H * W                      # 4096
MODES = 64
NCORES = 8
BS = B // NCORES               # 4 samples per core
LC = L // 128                  # 32 l-chunks of 128


def build_program(reps=1, mode="full"):
    """Build the per-core Bass program (same program on all 8 cores).

    reps>1 repeats the whole pipeline (same inputs/outputs) so device
    time per rep can be measured as a wall-clock delta, amortizing the
    axon dispatch overhead.  mode: "full" | "dmaonly" (loads+stores, no
    compute) | "noadd" (skip the residual add; store enh directly) —
    profiling variants only."""
    nc = bass.Bass()
    DT = FP8 if USE_FP8 else BF16

    xt_d = nc.dram_tensor("xt", [128, LC, BS * 128], DT, kind="ExternalInput")
    xf_d = nc.dram_tensor("xf", [BS, 128, L], F32, kind="ExternalInput")
    wr_d = nc.dram_tensor("wr", [128, MODES, 128], DT, kind="ExternalInput")
    wi_d = nc.dram_tensor("wi", [128, MODES, 128], DT, kind="ExternalInput")
    fb_d = nc.dram_tensor("fb", [128, LC, 128], DT, kind="ExternalInput")
    gb_d = nc.dram_tensor("gb", [128, L], DT, kind="ExternalInput")
    id_d = nc.dram_tensor("idn", [128, 128], BF16, kind="ExternalInput")
    out_d = nc.dram_tensor("out", [BS, 128, L], F32, kind="ExternalOutput")

    with tile.TileContext(nc) as tc:
        with (
            tc.tile_pool(name="consts", bufs=1) as consts,
            tc.tile_pool(name="xstage", bufs=2) as xstage,
            tc.tile_pool(name="outp", bufs=4) as outp,
            tc.tile_pool(name="ps_x", bufs=2, space="PSUM") as ps_x,
            tc.tile_pool(name="ps_y", bufs=1, space="PSUM") as ps_y,
            tc.tile_pool(name="ps_t", bufs=1, space="PSUM") as ps_t,
            tc.tile_pool(name="ps_e", bufs=2, space="PSUM") as ps_e,
        ):
            for _rep in range(reps):
                # ---- inputs (loaded per rep so reps-delta timing is
                # faithful to the single-shot kernel) -------------------
                xt = consts.tile([128, LC, BS * 128], DT, tag="xt")
                fb = consts.tile([128, LC, 128], DT, tag="fb")
                wr = consts.tile([128, MODES, 128], DT, tag="wr")
                wi = consts.tile([128, MODES, 128], DT, tag="wi")
                gb = consts.tile([128, L], DT, tag="gb")
                idn = consts.tile([128, 128], BF16, tag="idn")
                xf = consts.tile([128, BS, L], F32, tag="xf")

                nc.sync.dma_start(xt[:], xt_d[:])
                nc.sync.dma_start(fb[:], fb_d[:])
                nc.sync.dma_start(wr[:], wr_d[:])
                nc.sync.dma_start(wi[:], wi_d[:])
                nc.sync.dma_start(gb[:], gb_d[:])
                nc.sync.dma_start(idn[:], id_d[:])
                for b in range(BS):
                    nc.sync.dma_start(xf[:, b, :], xf_d[b])

                if mode == "dmaonly":
                    for b in range(BS):
                        for n in range(L // 512):
                            ob = outp.tile([128, 512], F32, tag="ob")
                            nc.vector.tensor_copy(
                                ob[:], xf[:, b, n * 512:(n + 1) * 512])
                            nc.sync.dma_start(
                                out_d[b][:, n * 512:(n + 1) * 512], ob[:])
                    continue
                # ---- stage 2 moving operands --------------------------
                # xcat  cols m*8+k : k<4 -> Xr[b=k],   k>=4 -> Xi[b=k-4]
                # xcat2 cols m*8+k : k<4 -> -Xi[b=k],  k>=4 -> Xr[b=k-4]
                xcat = xstage.tile([128, MODES, 8], DT, tag="xcat")
                xcat2 = xstage.tile([128, MODES, 8], DT, tag="xcat2")
                # ycat [o, b, mh] bf16 staging for the Y transposes
                ycat = xstage.tile([128, BS, 128], BF16, tag="ycat")
                # yt [mh, b, o] = stage-3 stationary operand
                yt = xstage.tile([128, BS, 128], DT, tag="yt")

                # ---- stage 1: forward DFT, per local sample ----------
                for b in range(BS):
                    xps = ps_x.tile([128, 128], F32, tag="xps")
                    for lc in range(LC):
                        nc.tensor.matmul(
                            xps[:],
                            xt[:, lc, b * 128:(b + 1) * 128],
                            fb[:, lc, :],
                            start=(lc == 0),
                            stop=(lc == LC - 1),
                        )
                    # drain [i, 128] psum -> fp8 stage-2 operand layouts
                    # (DVE: ACT's activation-copy is ~5x slower per element
                    # and these sit on the stage1->stage2 critical path)
                    nc.vector.tensor_copy(xcat[:, :, b], xps[:, 0:MODES])
                    nc.vector.tensor_copy(xcat[:, :, 4 + b], xps[:, MODES:128])
                    nc.vector.tensor_copy(xcat2[:, :, 4 + b], xps[:, 0:MODES])
                    nc.scalar.mul(xcat2[:, :, b], xps[:, MODES:128], -1.0)

                # ---- stage 2: per-mode complex channel mix -----------
                # ycomb cols m*8+k : k<4 -> Yr[b=k], k>=4 -> Yi[b=k-4]
                ycomb = ps_y.tile([128, MODES, 8], F32, tag="ycomb")
                for m in range(MODES):
                    # Yr = Wr@Xr - Wi@Xi ; Yi = Wr@Xi + Wi@Xr
                    nc.tensor.matmul(
                        ycomb[:, m, :], wr[:, m, :], xcat[:, m, :],
                        start=True, stop=False,
                    )
                    nc.tensor.matmul(
                        ycomb[:, m, :], wi[:, m, :], xcat2[:, m, :],
                        start=False, stop=True,
                    )

                # ---- Y -> [mh, o] per sample (PE transpose) ----------
                for b in range(BS):
                    nc.vector.tensor_scalar_mul(
                        ycat[:, b, 0:MODES], ycomb[:, :, b], SF_Y)
                    nc.vector.tensor_scalar_mul(
                        ycat[:, b, MODES:128], ycomb[:, :, 4 + b], SF_Y)
                for b in range(BS):
                    pst = ps_t.tile([128, 128], BF16, tag="pst")
                    nc.tensor.transpose(pst[:], ycat[:, b, :], idn[:])
                    nc.vector.tensor_copy(yt[:, b, :], pst[:])

                # ---- stage 3: inverse DFT + residual add -------------
                NW = 1024
                for b in range(BS):
                    for n in range(L // NW):
                        enh = ps_e.tile([128, NW], F32, tag="enh")
                        for h in range(NW // 512):
                            lo = n * NW + h * 512
                            nc.tensor.matmul(
                                enh[:, h * 512:(h + 1) * 512],
                                yt[:, b, :], gb[:, lo:lo + 512],
                                start=True, stop=True,
                            )
                        ob = outp.tile([128, NW], F32, tag="ob")
                        if mode == "noadd":
                            nc.vector.tensor_scalar_mul(ob[:], enh[:], SF_OUT)
                        else:
                            # out = enh*descale + feature, one DVE op
                            nc.vector.scalar_tensor_tensor(
                                ob[:], enh[:], SF_OUT,
                                xf[:, b, n * NW:(n + 1) * NW],
                                mybir.AluOpType.mult, mybir.AluOpType.add,
                            )
                        nc.sync.dma_start(
                            out_d[b][:, n * NW:(n + 1) * NW], ob[:])

    split_multi_waits(nc)
    return nc


def split_multi_waits(nc, max_waits=1):
    """The walrus in this container only accepts one sync-wait per
    instruction; move extras onto injected NoOps on the same engine."""
    cnt = 0
    for fn in nc.m.functions:
        for bb in fn.blocks:
            out = []
            for inst in bb.instructions:
                si = inst.sync_info
                if si is not None and si.on_wait and len(si.on_wait) > max_waits:
                    waits = list(si.on_wait)
                    for w in waits[:-max_waits]:
                        cnt += 1
                        nop = mybir.InstNoOp(
                            name=f"xsplitwait_{cnt}", ins=[], outs=[],
                            sync_info=mybir.SyncInfo(on_wait=[w], on_update=[]),
                        )
                        nop.engine = inst.engine
                        out.append(nop)
                    si.on_wait = waits[-max_waits:]
                    inst.sync_info = si
                out.append(inst)
            bb.instructions[:] = out
    return cnt


def make_host_inputs(feature, weights1_real, weights1_img):
    """Host-side prep: DFT bases, weight re-layout, per-core shards."""
    bf16 = ml_dtypes.bfloat16
    dt_np = ml_dtypes.float8_e3m4 if USE_FP8 else bf16
    sf_x = SF_X if USE_FP8 else 1.0
    sf_w = SF_W if USE_FP8 else 1.0
    x = np.ascontiguousarray(feature.reshape(B, C, L))

    lv = np.arange(L, dtype=np.float64)
    mv = np.arange(MODES, dtype=np.float64)
    theta = 2.0 * np.pi * np.outer(lv, mv) / L            # (L, 64)
    F = np.concatenate([np.cos(theta), -np.sin(theta)], axis=1)   # (L, 128)
    a = np.full(MODES, 2.0 / L)
    a[0] = 1.0 / L
    G = np.concatenate(
        [a[:, None] * np.cos(theta.T), -(a[:, None]) * np.sin(theta.T)], axis=0
    )                                                     # (128, L)

    # fb[p, c, m] = F[c*128+p, m] (scaled so |X| fits fp8 range)
    fb = np.ascontiguousarray(
        (F * sf_x).reshape(LC, 128, 128).transpose(1, 0, 2)
    ).astype(dt_np)
    # gb carries its own fp8-range scale; SF_OUT undoes everything at the end
    gb = np.ascontiguousarray(G * SF_G).astype(dt_np)

    # wr/wi[i, m, o] from (i, o, m, 1)
    wr = np.ascontiguousarray(
        weights1_real[..., 0].transpose(0, 2, 1) * sf_w
    ).astype(dt_np)
    wi = np.ascontiguousarray(
        weights1_img[..., 0].transpose(0, 2, 1) * sf_w
    ).astype(dt_np)

    idn = np.eye(128, dtype=np.float32).astype(bf16)

    in_maps = []
    for c in range(NCORES):
        xc = x[c * BS:(c + 1) * BS]                       # (4, 128, L)
        # xt[p, lc, b*128+i] = x[b, i, lc*128+p]
        xtc = xc.transpose(2, 0, 1).reshape(L, BS * 128)  # (L, 512)
        xt = np.ascontiguousarray(
            xtc.reshape(LC, 128, BS * 128).transpose(1, 0, 2)
        ).astype(dt_np)
        in_maps.append({
            "xt": xt,
            "xf": np.ascontiguousarray(xc, dtype=np.float32),
            "wr": wr,
            "wi": wi,
            "fb": fb,
            "gb": gb,
            "idn": idn,
        })
    return in_maps


_CACHE = {}


def get_program(reps=1, mode="full"):
    key = ("nc", reps, mode)
    if key not in _CACHE:
        _CACHE[key] = build_program(reps, mode)
    return _CACHE[key]


def kernel(feature, weights1_real, weights1_img):
    feature = np.asarray(feature, dtype=np.float32)
    weights1_real = np.asarray(weights1_real, dtype=np.float32)
    weights1_img = np.asarray(weights1_img, dtype=np.float32)

    nc = get_program()
    in_maps = make_host_inputs(feature, weights1_real, weights1_img)
    res = run_bass_kernel_spmd(nc, in_maps, list(range(NCORES)))
    out = np.concatenate([res.results[c]["out"] for c in range(NCORES)], axis=0)
    return out.reshape(B, C, H, W).astype(np.float32)
